# revision 1
# baseline (speedup 1.0000x reference)
"""Bispectrum loss kernel for Trainium2 (8 NeuronCores, SPMD).

Host-side: reflect-pad, im2col of only-valid STFT frames, even split across
8 cores (padded with identical pred/target frames -> zero loss contribution).
Device-side: windowed DFT as float32r matmul (full-rate), sign-free half-angle
atan2 (tau = arctan(b/(r+a)) with clamp; +-pi/2 flips absorbed mod pi by the
wrap), fused gathered bispectrum sums, L1 reductions with ACT Abs+accum.
Final scalar assembled on host from per-core partial sums.
"""

import hashlib
import os
import numpy as np
from numpy.lib.stride_tricks import as_strided

N_FFT = 512
HOP = 160
F = 257  # N_FFT//2 + 1
FB = 256  # main elementwise block (bins 0..255); bin 256 handled via scalars
NW = 258  # matmul output cols (fp32r needs even N); col 257 is zero-padding
NCORES = 8
PAD = N_FFT // 2

_prog_cache: dict[tuple, object] = {}
_runner_cache: dict[tuple, object] = {}
_input_cache: dict = {"key": None}
last_results = None
last_exec_info = None


def _cfg():
    return {
        "gps_msq": os.environ.get("BISPEC_GPS_MSQ", "1") == "1",
        "gps_clamp": os.environ.get("BISPEC_GPS_CLAMP", "1") == "1",
        "gps_u": os.environ.get("BISPEC_GPS_U", "1") == "1",
        "reps": int(os.environ.get("BISPEC_REPS", "1")),
    }


def _build_program(VC: int, cfg_key: tuple):
    """Per-core Bass program for VC frames (mult of 128)."""
    import concourse.bacc as bacc
    import concourse.mybir as mybir
    import concourse.tile as tile

    cfg = dict(zip(("gps_msq", "gps_clamp", "gps_u", "reps"), cfg_key))
    reps = cfg["reps"]
    f32 = mybir.dt.float32
    f32r = mybir.dt.float32r
    h16 = mybir.dt.float16
    A = mybir.AluOpType
    AF = mybir.ActivationFunctionType
    PI = float(np.pi)

    nT = VC // 128
    CH = max(1, (nT + 2) // 3)   # phase B/C chunk size (3 pipeline chunks)
    nc = bacc.Bacc("TRN2")

    xp_d = nc.dram_tensor("xp", [512, VC], f32r, kind="ExternalInput")
    xt_d = nc.dram_tensor("xt", [512, VC], f32r, kind="ExternalInput")
    w_d = nc.dram_tensor("w", [128, 2, 4, NW], f32r, kind="ExternalInput")
    oth_d = nc.dram_tensor("o_th", [128, nT], f32, kind="ExternalOutput")
    ol_d = nc.dram_tensor("o_l", [128, nT], f32, kind="ExternalOutput")

    # [512, VC] -> [p=128, c=4, VC] with k = 128*c + p
    xs = [
        xp_d[:].rearrange("(c p) v -> p c v", p=128),
        xt_d[:].rearrange("(c p) v -> p c v", p=128),
    ]

    def tt_sbuf(engine_gps, out, in0, in1, op):
        if engine_gps:
            nc.gpsimd.tensor_tensor(out, in0, in1, op)
        else:
            nc.vector.tensor_tensor(out, in0, in1, op)

    AX = mybir.AxisListType

    with tile.TileContext(nc) as tc:
        with tc.tile_pool(name="const", bufs=1) as const, \
             tc.tile_pool(name="frames", bufs=3) as frp, \
             tc.tile_pool(name="work", bufs=3) as work, \
             tc.tile_pool(name="store", bufs=1) as store, \
             tc.tile_pool(name="psum", bufs=2, space="PSUM") as psum:

            wsb = const.tile([128, 2, 4, NW], f32r)
            nc.sync.dma_start(wsb[:], w_d[:])
            eps_lm = const.tile([128, 1], f32)
            nc.vector.memset(eps_lm[:], 1e-35)

            red_th = store.tile([128, nT], f32, tag="red_th")
            red_l = store.tile([128, nT], f32, tag="red_l")
            # [plane(a=0,b=1), sig(p=0,t=1), col]
            ab_st = store.tile([128, 2, nT, 2, NW], h16, tag="ab_st")
            lm_st = store.tile([128, nT, 2, FB], h16, tag="lm_st")
            lmx_st = store.tile([128, nT, 2], f32, tag="lmx_st")
            u_st = store.tile([128, nT, 2, FB], h16, tag="u_st")
            q4_st = store.tile([128, nT, 2, 2], f32, tag="q4_st")
            dl_st = store.tile([128, nT, F], h16, tag="dl_st")
            dth_st = store.tile([128, nT, F], h16, tag="dth_st")
            r16 = store.tile([128, nT, 2, FB], h16, tag="r16")
            dfl = store.tile([128, nT, 2, FB], f32, tag="dfl")
            em = store.tile([128, nT, 2, FB], f32, tag="em")
            dd_all = store.tile([128, nT, FB], h16, tag="dd_all")
            db_sc = store.tile([128, nT, 128], h16, tag="db_sc")
            d256 = store.tile([128, nT], f32, tag="d256")
            tau = store.tile([128, nT, 2, FB], h16, tag="tau")
            stt_ = store.tile([128, nT, 2, FB], h16, tag="stt_")
            sb1 = store.tile([128, nT, 2, 128], h16, tag="sb1")
            wvt = store.tile([128, nT, 2, FB], h16, tag="wvt")
            sg = store.tile([128, nT, 2, 2], f32, tag="sg")

            import contextlib
            rep_ctx = tc.For_i(0, reps) if reps > 1 else contextlib.nullcontext(0)
            with rep_ctx:
                for _rep_once in range(1):
                    # ---- phase A (ACT set: natural_log): DFT, squares, logs, mag
                    for i in range(nT):
                        frs = []
                        for r in range(2):
                            fr = frp.tile([128, 4, 128], f32r, tag=f"fr{r}",
                                          name=f"fr{r}")
                            nc.sync.dma_start(fr[:], xs[r][:, :, i * 128:(i + 1) * 128])
                            frs.append(fr)
                        # psum banks: 0 = a_p, 1 = b_p, 2 = a_t, 3 = b_t
                        pst = psum.tile([128, 4, 512], f32, tag="pst")
                        for c in range(4):
                            for r in range(2):
                                for plane in range(2):
                                    nc.tensor.matmul(
                                        pst[:, 2 * r + plane, 0:NW],
                                        frs[r][:, c, :], wsb[:, plane, c, :],
                                        start=(c == 0), stop=(c == 3),
                                        skip_group_check=True)

                        nc.scalar.activation(
                            ab_st[:, :, i].rearrange("p a s f -> p s a f")[:, :, :, 0:F],
                            pst[:, :, 0:F].rearrange("p (s a) f -> p s a f", s=2),
                            AF.Copy)
                        sqa = work.tile([128, 2, FB], h16, tag="sqa")
                        nc.vector.tensor_tensor(sqa[:], ab_st[:, 0, i, :, 0:FB],
                                                ab_st[:, 0, i, :, 0:FB], A.mult)
                        sqb = work.tile([128, 2, FB], h16, tag="sqb")
                        nc.vector.tensor_tensor(sqb[:], ab_st[:, 1, i, :, 0:FB],
                                                ab_st[:, 1, i, :, 0:FB], A.mult)
                        msq = work.tile([128, 2, FB], h16, tag="msq")
                        tt_sbuf(cfg["gps_msq"], msq[:], sqa[:], sqb[:], A.add)
                        nc.scalar.activation(lm_st[:, i], msq[:],
                                             AF.Ln, bias=eps_lm[:])

                    # mag loss, batched across all tiles
                    sq256 = work.tile([128, nT, 2], f32, tag="sq256")
                    nc.vector.tensor_tensor(sq256[:], ab_st[:, 0, :, :, 256],
                                            ab_st[:, 0, :, :, 256], A.mult)
                    nc.scalar.activation(lmx_st[:], sq256[:], AF.Ln, bias=eps_lm[:])
                    nc.vector.tensor_tensor(dd_all[:], lm_st[:, :, 0], lm_st[:, :, 1],
                                            A.subtract)
                    nc.vector.tensor_tensor(d256[:], lmx_st[:, :, 0], lmx_st[:, :, 1],
                                            A.subtract)
                    nc.vector.scalar_tensor_tensor(
                        dl_st[:, :, 0:128], dd_all[:, :, 0:128], 2.0,
                        dd_all[:, :, 0:FB:2], A.mult, A.add)
                    nc.vector.scalar_tensor_tensor(
                        db_sc[:], dd_all[:, :, 128:256], 0.0,
                        dd_all[:, :, 128:0:-1], A.add, A.add)
                    nc.vector.tensor_tensor(
                        dl_st[:, :, 128:256], db_sc[:],
                        d256[:, :, None].broadcast_to([128, nT, 128]), A.add)
                    nc.vector.scalar_tensor_tensor(
                        dl_st[:, :, 256], d256[:], 2.0, dd_all[:, :, 0],
                        A.mult, A.add)
                    nc.vector.tensor_reduce(red_l[:], dl_st[:], AX.X, A.add,
                                            apply_absolute_value=True)

                    tc.no_sync_barrier()

                    # ---- phase B (ACT set: exp): r = |S|, d = r+a, em = 1/d, u
                    nc.scalar.activation(sg[:], ab_st[:, 0, :, :, 0:F:256], AF.Sign)
                    nc.vector.tensor_scalar(
                        q4_st[:].rearrange("p t s c -> p (t s c)"),
                        sg[:].rearrange("p t s c -> p (t s c)"),
                        -PI / 4, PI / 4, A.mult, A.add)
                    for t0 in range(0, nT, CH):
                        t1 = min(t0 + CH, nT)
                        a3 = ab_st[:, 0, t0:t1].rearrange(
                            "p t s f -> p (t s) f")[:, :, 0:FB]
                        b3 = ab_st[:, 1, t0:t1].rearrange(
                            "p t s f -> p (t s) f")[:, :, 0:FB]
                        r3 = r16[:, t0:t1].rearrange("p t s f -> p (t s) f")
                        d3 = dfl[:, t0:t1].rearrange("p t s f -> p (t s) f")
                        e3 = em[:, t0:t1].rearrange("p t s f -> p (t s) f")
                        u3 = u_st[:, t0:t1].rearrange("p t s f -> p (t s) f")
                        nc.scalar.activation(r16[:, t0:t1], lm_st[:, t0:t1],
                                             AF.Exp, scale=0.5)
                        nc.vector.scalar_tensor_tensor(d3, a3, 0.0, r3,
                                                       A.add, A.add)
                        if cfg["gps_clamp"]:
                            nc.gpsimd.tensor_scalar(d3, d3, 1e-12, None, A.max)
                        else:
                            nc.vector.tensor_scalar(d3, d3, 1e-12, None, A.max)
                        nc.vector.reciprocal_approx_fast(
                            em[:, t0:t1].rearrange("p t s f -> p (t s f)"),
                            dfl[:, t0:t1].rearrange("p t s f -> p (t s f)"))
                        tt_sbuf(cfg["gps_u"], u3, b3, e3, A.mult)

                    tc.no_sync_barrier()

                    # ---- phase C (ACT set: trig): angles, wrap, phase loss
                    for t0 in range(0, nT, CH):
                        t1 = min(t0 + CH, nT)
                        g = t1 - t0
                        tau3 = tau[:, t0:t1].rearrange("p t s f -> p (t s) f")
                        st3 = stt_[:, t0:t1].rearrange("p t s f -> p (t s) f")
                        nc.scalar.activation(tau[:, t0:t1], u_st[:, t0:t1], AF.Arctan)
                        nc.vector.tensor_copy(
                            tau3[:, :, 0:1],
                            q4_st[:, t0:t1].rearrange("p t s c -> p (t s) c")[:, :, 0:1])
                        nc.vector.scalar_tensor_tensor(
                            st3[:, :, 0:128], tau3[:, :, 0:128], 2.0,
                            tau3[:, :, 0:FB:2], A.mult, A.subtract)
                        nc.vector.scalar_tensor_tensor(
                            sb1[:, t0:t1].rearrange("p t s f -> p (t s) f"),
                            tau3[:, :, 128:256], 0.0,
                            tau3[:, :, 128:0:-1], A.add, A.add)
                        nc.vector.tensor_tensor(
                            st3[:, :, 128:256],
                            sb1[:, t0:t1].rearrange("p t s f -> p (t s) f"),
                            q4_st[:, t0:t1].rearrange(
                                "p t s c -> p (t s) c")[:, :, 1:2].broadcast_to(
                                [128, 2 * g, 128]), A.subtract)
                        nc.vector.add_range_wrap(
                            wvt[:, t0:t1].rearrange("p t s f -> p (t s f)"),
                            stt_[:, t0:t1].rearrange("p t s f -> p (t s f)"),
                            0.0, PI / 2, PI)
                        nc.gpsimd.tensor_tensor(dth_st[:, t0:t1, 0:FB],
                                                wvt[:, t0:t1, 0], wvt[:, t0:t1, 1],
                                                A.subtract)
                        nc.vector.tensor_copy(dth_st[:, t0:t1, 256:257],
                                              dth_st[:, t0:t1, 0:1])
                        nc.vector.tensor_reduce(red_th[:, t0:t1], dth_st[:, t0:t1],
                                                AX.X, A.add,
                                                apply_absolute_value=True)
                    tc.no_sync_barrier()

            nc.sync.dma_start(oth_d[:], red_th[:])
            nc.sync.dma_start(ol_d[:], red_l[:])

    nc.compile()
    return nc


_w_cache: dict = {}


def _weights(window: np.ndarray) -> np.ndarray:
    """[128, 2, 4, 258] fp32: windowed rDFT, re/im planes, 4 K-chunks."""
    key = hashlib.blake2b(window.tobytes(), digest_size=8).hexdigest()
    w = _w_cache.get(key)
    if w is not None:
        return w
    k = np.arange(N_FFT, dtype=np.float64)
    f = np.arange(F, dtype=np.float64)
    ang = 2.0 * np.pi * (np.outer(k, f) % N_FFT) / N_FFT
    win = window.astype(np.float64)
    wre = np.cos(ang) * win[:, None]
    wim = -np.sin(ang) * win[:, None]
    wim[:, 0] = 0.0
    wim[:, 256] = 0.0
    wre4 = wre.reshape(4, 128, F).transpose(1, 0, 2)   # [p, c, f]
    wim4 = wim.reshape(4, 128, F).transpose(1, 0, 2)
    w = np.zeros((128, 2, 4, NW), dtype=np.float32)
    w[:, 0, :, 0:F] = wre4
    w[:, 1, :, 0:F] = wim4
    w = np.ascontiguousarray(w)
    _w_cache[key] = w
    return w


def _frames_valid(x: np.ndarray, idxb: np.ndarray, idxt: np.ndarray) -> np.ndarray:
    """Gather valid [V, 512] frames from reflect-padded signals."""
    xp = np.pad(x, ((0, 0), (PAD, PAD)), mode="reflect")
    xp = np.ascontiguousarray(xp)
    B, Lp = xp.shape
    T = 1 + x.shape[1] // HOP
    sb, se = xp.strides
    fr = as_strided(xp, (B, T, N_FFT), (sb, HOP * se, se))
    return fr[idxb, idxt]  # [V, 512] copy


def prepare(preds, targets, window, lengths):
    """Host prep: returns (compiled bass program, per-core in_maps, nT)."""
    preds = np.asarray(preds, dtype=np.float32)
    targets = np.asarray(targets, dtype=np.float32)
    window = np.asarray(window, dtype=np.float32)
    lengths = np.asarray(lengths).astype(np.int64)

    B, L = preds.shape
    T = 1 + L // HOP
    n_valid = 1 + lengths // HOP                     # [B]
    mask = np.arange(T)[None, :] < n_valid[:, None]
    idxb, idxt = np.nonzero(mask)
    V = len(idxb)

    P_all = _frames_valid(preds, idxb, idxt)         # [V, 512]
    T_all = _frames_valid(targets, idxb, idxt)

    Vc = -(-V // NCORES)
    nT = max(1, -(-Vc // 128))
    VCpad = nT * 128

    w = _weights(window)
    cfg = _cfg()
    cfg_key = tuple(cfg.values())
    key = (VCpad, cfg_key)
    nc = _prog_cache.get(key)
    if nc is None:
        nc = _build_program(VCpad, cfg_key)
        _prog_cache[key] = nc

    pad_frame = P_all[0]
    in_maps = []
    for c in range(NCORES):
        lo, hi = c * Vc, min((c + 1) * Vc, V)
        n = max(0, hi - lo)
        xp_c = np.empty((VCpad, N_FFT), dtype=np.float32)
        xt_c = np.empty((VCpad, N_FFT), dtype=np.float32)
        if n:
            xp_c[:n] = P_all[lo:hi]
            xt_c[:n] = T_all[lo:hi]
        if n < VCpad:
            xp_c[n:] = pad_frame
            xt_c[n:] = pad_frame
        in_maps.append({
            "xp": np.ascontiguousarray(xp_c.T),
            "xt": np.ascontiguousarray(xt_c.T),
            "w": w,
        })
    return nc, in_maps, nT, key


def _get_runner(nc, key):
    """Persistent jitted shard_map executor for program `nc` (8 cores).

    Returns run(concat_inputs_list) -> list of per-core output dicts.
    Mirrors bass2jax.run_bass_via_pjrt but caches the jitted callable.
    """
    cached = _runner_cache.get(key)
    if cached is not None:
        return cached

    import jax
    import concourse.mybir as mybir
    from concourse import bass2jax
    from concourse.bass2jax import (_bass_exec_p, install_neuronx_cc_hook,
                                    partition_id_tensor)
    from jax.experimental.shard_map import shard_map
    from jax.sharding import Mesh, PartitionSpec

    install_neuronx_cc_hook()
    partition_name = nc.partition_id_tensor.name if nc.partition_id_tensor else None

    in_names, out_names, out_avals, zero_outs = [], [], [], []
    for alloc in nc.m.functions[0].allocations:
        if not isinstance(alloc, mybir.MemoryLocationSet):
            continue
        name = alloc.memorylocations[0].name
        if alloc.kind == "ExternalInput":
            if name != partition_name:
                in_names.append(name)
        elif alloc.kind == "ExternalOutput":
            shape = tuple(alloc.tensor_shape)
            dtype = mybir.dt.np(alloc.dtype)
            out_names.append(name)
            out_avals.append(jax.core.ShapedArray(shape, dtype))
            zero_outs.append(np.zeros(shape, dtype))
    n_params = len(in_names)
    n_outs = len(out_avals)
    all_names = list(in_names) + list(out_names)
    if partition_name is not None:
        all_names.append(partition_name)
    donate = tuple(range(n_params, n_params + n_outs))

    def _body(*args):
        operands = list(args)
        if partition_name is not None:
            operands.append(partition_id_tensor())
        outs = _bass_exec_p.bind(
            *operands,
            out_avals=tuple(out_avals),
            in_names=tuple(all_names),
            out_names=tuple(out_names),
            lowering_input_output_aliases=(),
            sim_require_finite=True,
            sim_require_nnan=True,
            nc=nc,
        )
        return tuple(outs)

    devices = jax.devices()[:NCORES]
    mesh = Mesh(np.asarray(devices), ("core",))
    in_specs = (PartitionSpec("core"),) * (n_params + n_outs)
    out_specs = (PartitionSpec("core"),) * n_outs
    sharded = jax.jit(
        shard_map(_body, mesh=mesh, in_specs=in_specs, out_specs=out_specs,
                  check_rep=False),
        donate_argnums=donate, keep_unused=True)

    runner = {
        "sharded": sharded, "in_names": in_names, "out_names": out_names,
        "out_avals": out_avals, "zero_shapes": [(z.shape, z.dtype) for z in zero_outs],
    }
    _runner_cache[key] = runner
    return runner


def _run(nc, in_maps, key):
    import jax
    runner = _get_runner(nc, key)
    in_names = runner["in_names"]
    out_names = runner["out_names"]
    out_avals = runner["out_avals"]

    # fingerprint inputs so repeat calls reuse on-device arrays
    h = hashlib.blake2b(digest_size=16)
    for m in in_maps:
        for name in in_names:
            h.update(np.ascontiguousarray(m[name]).tobytes())
    fp = (key, h.hexdigest())

    if _input_cache.get("key") == fp:
        dev_inputs = _input_cache["dev_inputs"]
    else:
        concat_in = [
            np.concatenate([np.asarray(in_maps[c][name]) for c in range(NCORES)],
                           axis=0)
            for name in in_names
        ]
        dev_inputs = jax.device_put(concat_in)
        dev_inputs = [x.block_until_ready() for x in dev_inputs]
        _input_cache["key"] = fp
        _input_cache["dev_inputs"] = dev_inputs

    concat_zeros = [np.zeros((NCORES * s[0], *s[1:]), d)
                    for (s, d) in runner["zero_shapes"]]
    import time
    t0 = time.time()
    out_arrs = runner["sharded"](*dev_inputs, *concat_zeros)
    out_np = [np.asarray(o) for o in out_arrs]
    t1 = time.time()
    global last_exec_info
    last_exec_info = {"exec_wall_s": t1 - t0}
    return [
        {name: out_np[i].reshape(NCORES, *out_avals[i].shape)[c]
         for i, name in enumerate(out_names)}
        for c in range(NCORES)
    ]


def kernel(preds, targets, window, lengths):
    global last_results
    nc, in_maps, nT, key = prepare(preds, targets, window, lengths)
    results = _run(nc, in_maps, key)
    last_results = results

    th = 0.0
    lg = 0.0
    for r in results:
        th += r["o_th"].astype(np.float64).sum()
        lg += r["o_l"].astype(np.float64).sum()
    loss = 2.0 * th + 0.5 * lg
    return np.array(loss, dtype=np.float32)

